# revision 44
# baseline (speedup 1.0000x reference)
"""Trainium2 Bass kernel for DiffDimDotProductAttention.

reference computation:
    q = queries @ W_q                      [B, LQ, DKEY]
    scores = q @ keys^T / sqrt(DKEY)       [B, LQ, LK]
    scores = where(arange(LK) < valid_len, scores, -1e6)
    attn = softmax(scores, axis=-1)
    out = attn @ values                    [B, LQ, DV]

Sharding: every core gets 2048 query rows. The schedule is a per-core list of
SLOTS, identical in shape across cores (SPMD: one program), but bound to
different (batch, row-range) data per core. A batch with nb active key blocks
contributes 8 x 256-row slices; when two batches share the same nb, each core
takes a single 512-row slice of ONE of them instead of 256 rows of both, so
that batch's K/V stream is fetched once per core instead of twice. For the
reference valid_lens this cuts K/V HBM traffic by 25% and lets the scores
matmuls stream 512 columns.

Per slot the device runs two phases (P = exp(masked scores/sqrt(d)) kept in
SBUF between them):
    phase 1: for each key block s: scoresT[s,l] chain over kc, then
             P[s,:] = exp(scores/32 + mask_col) on the scalar engine; the
             otherwise-idle gpsimd engine keeps a running block-sum
             psum = sum_s P[s,:] as the exps land
    phase 2: for each 128-row tile t: ONE 1-col rowsum matmul
             rowsum[t] = psum[:,t]^T @ ones first (so the reciprocal
             overlaps the whole chain), then chain over s of
             out[t] += P[s,t]^T @ V[s],
             then normalize (vector) and DMA the f16 tile out.
The gpsimd block-sum replaces the per-(s,t) 1-col rowsum matmuls of the
earlier version: ~120 fewer PE issue slots (~27ns each) and an earlier
reciprocal -> ~4.5us off the matmul stream.  The f16 block-sum adds
O(sqrt(nb)*eps_f16) ~ 1e-3 relative error to the denominators, well
inside the 2e-2 gate (measured total 5.4e-4).
Phase 2 needs only 2 PSUM banks per tile (double-buffered), so scores can
stream 512 wide, and each tile's normalize+store overlaps the next tile's
matmul chain.  Phases are software-pipelined depth-1 across slots (phase 1
of slot i+1 is emitted before phase 2 of slot i) so phase-2 chains never
wait on exp outputs, and the final tile stores in halves on two queues to
shorten the drain.  Softmax skips the max-subtraction: scores ~ N(0,1),
exp() cannot overflow, and softmax is shift invariant.  Masked lanes get
bias -1e6 and underflow to exactly 0.0.

DMA discipline (the hard-won part): every DMA issue queue (SP / Pool /
Act) has 8 completion semaphores, and a reader of DMA #p conservatively
waits for every use of semaphore p%8 scheduled before it.  An un-gated
DMA gets hoisted by the tile scheduler and so delays the prologue's
readers by its own completion time.  Therefore each queue carries exactly
8 un-gated prologue DMAs (W_q chunk 0 FIRST -- the first projection
LDWEIGHTS waits on it and it must head the sync queue, not sit behind
two qtr chunks -- then qtr group 0, the other W_q chunks, mask, the
first kt blocks of slot 0), and every later DMA is dependency-gated --
by pool-buffer exhaustion (kt per-block tiles, ~12 in flight across the
16 DMA engines) or an explicit 1-column pre-touch copy tied to the first
projection's output (slot-0 kt tail, group-1 qtr BEFORE slot-0 V: the
g1 projection follows phase1(0) directly and its landing margin is
~zero, while V is consumed one phase later) or to each slot's first exp
(next slot's V).  All DRAM layouts are contiguous per partition.  Out
stores ride the sync/gpsimd queues, not scalar, so their in-queue waits
never delay the next slot's exps -- EXCEPT the last two slots' odd
tiles, which store via scalar (its exps are done by then): gpsimd is
software-DGE and its epilogue DRAIN burns ~5.4us on completion
detection, so its last DMA must land early enough for the drain to
overlap compute instead of binding the final barrier.

Measured (2.37GHz warm PE): 197.1-199.3us vs 203.0 for the session-start
baseline; stream is gapless within ~3us of the 1-col/cycle f16 floor.
The ~13us tail is dominated by a framework-fixed ~6.5us semaphore-
zeroing epilogue plus the final stores and barrier.  The device flips
between 2.4GHz and a throttled 2.0GHz P0 state run-to-run (median
512-col MM spacing 215ns vs 258ns -- test.py prints a probe); compare
timings only at matched clock.
"""

import math
import os
import sys

import numpy as np

DTYPE = os.environ.get("KERNEL_DTYPE", "f16")


def _ensure_paths():
    try:
        import concourse  # noqa: F401
        return
    except ImportError:
        pass
    for p in (
        "/root/.axon_site",
        "/root/.axon_site/_ro/trn_rl_repo",
        "/root/.axon_site/_ro/pypackages",
        "/opt/trn_rl_repo",
    ):
        if p not in sys.path:
            sys.path.append(p)
    import concourse  # noqa: F401


B, LQ, LK, DQ, DKEY, DV = 8, 2048, 2048, 1024, 1024, 1024
NCORES = 8
SLICE = LQ // NCORES          # 256 query rows per batch-slice
PB = 128                      # partition block
NKC = DQ // PB                # 8 contraction blocks for the projections
NSB = LK // PB                # 16 key blocks
GW = 4                        # key blocks per kT DMA group
NWARM = 13                    # HAM warm-up matmuls (cover until the first
                              # real chain's data lands ~13us with the
                              # wq0-first wave; longer would queue-block
                              # the real chains behind the warmup)
SCALE = 1.0 / math.sqrt(DKEY)
MASK_NEG = -1.0e6

_program_cache = {}
_last_results = None


def _np_store_dtype(dtype):
    if dtype == "bf16":
        import ml_dtypes
        return ml_dtypes.bfloat16
    if dtype == "f16":
        return np.float16
    return np.float32


def _build_schedule(nb):
    """Identical-across-cores slot schedule from per-batch block counts.

    Returns (slots, groups). Each slot: dict with
      w: 256 or 512 query rows;  nb: active key blocks
      batches: 1 batch (every core takes its w-slice of it) or 2 equal-nb
               batches (cores 0-3 take 512-row slices of the first, 4-7 of
               the second)
      qoff: column offset of this slot inside its projection group
      gi: projection group index
    groups: list of dicts {slots: [slot indices], W: total width, base: qtr
            column base (in per-kc units)}.
    """
    order = sorted(range(B), key=lambda i: (-nb[i], i))
    byval = {}
    for b in order:
        byval.setdefault(nb[b], []).append(b)
    slots = []
    for v, bs in byval.items():
        i = 0
        while len(bs) - i >= 2:
            slots.append({"w": 2 * SLICE, "nb": v, "batches": (bs[i], bs[i + 1])})
            i += 2
        if i < len(bs):
            slots.append({"w": SLICE, "nb": v, "batches": (bs[i],)})
    # 512-row slots first (their 1.7us/block chains halve the K/V demand
    # rate, easing the cold start, and buy prefetch runway for the rest),
    # then 256-row slots; nb descending within each, smallest slot drains.
    slots.sort(key=lambda s: (-s["w"], -s["nb"]))

    groups = []
    pend = None
    for si, s in enumerate(slots):
        if s["w"] == 2 * SLICE:
            groups.append([si])
        elif pend is None:
            pend = si
        else:
            groups.append([pend, si])
            pend = None
    if pend is not None:
        groups.append([pend])

    base = 0
    gmeta = []
    for gi, g in enumerate(groups):
        W = sum(slots[si]["w"] for si in g)
        off = 0
        for si in g:
            slots[si]["qoff"] = off
            slots[si]["gi"] = gi
            off += slots[si]["w"]
        gmeta.append({"slots": g, "W": W, "base": base})
        base += W

    # flat offsets for kt / vv / mk DRAM params and out tiles
    kto = vvo = mko = to = 0
    for s in slots:
        s["kt_off"], s["vv_off"], s["mk_off"], s["tile0"] = kto, vvo, mko, to
        kto += NKC * s["nb"] * PB
        vvo += s["nb"] * DV
        mko += s["nb"]
        to += s["w"] // PB
    return slots, gmeta, {"kt": kto, "vv": vvo, "mk": mko, "tiles": to,
                          "qtr": base}


def _sig(nb):
    slots, gmeta, _ = _build_schedule(nb)
    return tuple((s["w"], s["nb"]) for s in slots)


def _build_program(nb, dtype):
    _ensure_paths()
    import concourse.mybir as mybir
    import concourse.tile as tile
    from concourse import bacc

    f32 = mybir.dt.float32
    if dtype == "bf16":
        dt_s = mybir.dt.bfloat16
    elif dtype == "f16":
        dt_s = mybir.dt.float16
    else:
        dt_s = mybir.dt.float32
    Exp = mybir.ActivationFunctionType.Exp
    Copy = mybir.ActivationFunctionType.Copy

    slots, gmeta, tot = _build_schedule(nb)
    # kt pool depth = first slot's block count, so the second slot's kt
    # DMAs are pool-gated behind the first slot's chains (un-gated DMAs
    # would be hoisted and poison reused prologue semaphores)
    kt_bufs = min(12, max(8, slots[0]["nb"]))

    nc = bacc.Bacc("TRN2", target_bir_lowering=False, debug=False,
                   num_devices=NCORES)
    # layouts are contiguous per partition for single-DMA loads
    qtr_d = nc.declare_dram_parameter("qtr", [PB, NKC * tot["qtr"]], dt_s,
                                      isOutput=False)
    kt_d = nc.declare_dram_parameter("kt", [PB, tot["kt"]], dt_s,
                                     isOutput=False)
    vv_d = nc.declare_dram_parameter("vv", [PB, tot["vv"]], dt_s,
                                     isOutput=False)
    wq_d = nc.declare_dram_parameter("wq", [PB, NKC * DKEY], dt_s,
                                     isOutput=False)
    mk_d = nc.declare_dram_parameter("mk", [PB, tot["mk"]], f32,
                                     isOutput=False)
    out_d = nc.declare_dram_parameter("out", [tot["tiles"], PB, DV], dt_s,
                                      isOutput=True)

    MM = nc.tensor.matmul

    with tile.TileContext(nc) as tc, \
         tc.tile_pool(name="const", bufs=1) as constp, \
         tc.tile_pool(name="qtr", bufs=2) as qtrp, \
         tc.tile_pool(name="qt", bufs=2) as qtp, \
         tc.tile_pool(name="kt", bufs=kt_bufs) as ktp, \
         tc.tile_pool(name="vt", bufs=3) as vtp, \
         tc.tile_pool(name="pt", bufs=2) as ptp, \
         tc.tile_pool(name="sum", bufs=2) as sump, \
         tc.tile_pool(name="outs", bufs=3) as outsp, \
         tc.tile_pool(name="rcp", bufs=4) as rcpp, \
         tc.tile_pool(name="ps_a", bufs=2, space="PSUM") as psa, \
         tc.tile_pool(name="ps_o", bufs=2, space="PSUM") as pso, \
         tc.tile_pool(name="ps_r", bufs=2, space="PSUM") as psr:

        ENGS = [nc.sync, nc.gpsimd, nc.scalar]
        wq_sb = constp.tile([PB, NKC * DKEY], dt_s)
        mask_sb = constp.tile([PB, tot["mk"]], f32)

        # Each DMA issue queue (SP / Pool / Act) has 8 completion
        # semaphores; a reader of DMA #p conservatively waits for every use
        # of semaphore p%8 scheduled before it, so any un-gated DMA past
        # position 8 delays the prologue's readers by its own completion.
        # Hence: at most 8 un-gated DMAs per queue (qtr0 + wq + mask +
        # first kt blocks), and every later DMA is dependency-gated via a
        # 1-column pre-touch copy or pool-buffer exhaustion.
        g0 = gmeta[0]
        g0c = NKC * g0["W"]
        qtr0_sb = qtrp.tile([PB, g0c], dt_s, tag="qtr", name="qtr0")

        kt_pre = {}     # s -> prefetched kt tile of slot 0
        vv_tiles = {}   # si -> vv tile

        def dma_kt(si, s, eng, pre_src=None):
            kt_sb = ktp.tile([PB, NKC * PB], dt_s, tag="kt", name="ktb")
            if pre_src is not None:
                nc.vector.tensor_copy(kt_sb[:, 0:1], pre_src)
            base = slots[si]["kt_off"] + s * NKC * PB
            eng.dma_start(out=kt_sb, in_=kt_d[:, base:base + NKC * PB])
            return kt_sb

        def dma_vv(si, pre_src):
            slot = slots[si]
            nbv, vo = slot["nb"], slot["vv_off"]
            vv_sb = vtp.tile([PB, nbv * DV], dt_s, tag="vt", name="vt")
            if pre_src is not None:
                nc.vector.tensor_copy(vv_sb[:, 0:1], pre_src)
            for ci, c in enumerate(range(0, nbv, 2)):
                e = min(c + 2, nbv)
                eng = nc.sync if ci % 2 == 0 else nc.gpsimd
                eng.dma_start(out=vv_sb[:, c * DV:e * DV],
                              in_=vv_d[:, vo + c * DV:vo + e * DV])
            vv_tiles[si] = vv_sb

        # prologue wave: exactly 8 un-gated DMAs per queue (fresh
        # semaphores).  qtr0 (6 chunks) + wq (8 kco chunks) + mask + the
        # first 9 kt blocks of slot 0 fill the 24 slots.  wq chunk 0 is
        # emitted FIRST (before the qtr chunks) so it heads the sync
        # queue: the very first projection LDWEIGHTS waits on it, and
        # behind two qtr chunks it landed ~3.5us later than necessary.
        nb0 = slots[0]["nb"]

        def dma_wq(kco):
            a, b = kco * DKEY, (kco + 1) * DKEY
            ENGS[kco % 3].dma_start(out=wq_sb[:, a:b], in_=wq_d[:, a:b])

        dma_wq(0)
        for i in range(6):
            a, b = g0c * i // 6, g0c * (i + 1) // 6
            ENGS[i % 3].dma_start(out=qtr0_sb[:, a:b], in_=qtr_d[:, a:b])
        for kco in range(1, NKC):
            dma_wq(kco)
        nc.scalar.dma_start(out=mask_sb, in_=mk_d[:, :])
        u0 = min(nb0, 9)
        for s in range(u0):
            kt_pre[s] = dma_kt(0, s, ENGS[s % 3])

        ones_sb = constp.tile([PB, 1], dt_s)
        nc.vector.memset(ones_sb, 1.0)

        # HAM warm-up: throwaway matmuls on zeroed SBUF while the first
        # DMA wave lands, so real work starts at full PE activity
        warm_sb = constp.tile([PB, 512], dt_s, name="warm_sb")
        nc.vector.memset(warm_sb, 0.0)
        wps = psa.tile([PB, 512], f32, tag="ps_a", name="warmps")
        for i in range(NWARM):
            MM(wps, warm_sb[:, :PB], warm_sb, start=(i == 0),
               stop=(i == NWARM - 1))

        qt_tiles = {}
        qtr_tiles = {0: qtr0_sb}

        def prefetch_qtr(gi, pre_src):
            g = gmeta[gi]
            gb, gc = NKC * g["base"], NKC * g["W"]
            qtr_sb = qtrp.tile([PB, gc], dt_s, tag="qtr", name="qtr")
            if pre_src is not None:
                nc.vector.tensor_copy(qtr_sb[:, 0:1], pre_src)
            for i in range(4):
                a, b = gc * i // 4, gc * (i + 1) // 4
                eng = nc.sync if i % 2 == 0 else nc.gpsimd
                eng.dma_start(out=qtr_sb[:, a:b], in_=qtr_d[:, gb + a:gb + b])
            qtr_tiles[gi] = qtr_sb

        def qproj(gi):
            g = gmeta[gi]
            W = g["W"]
            if gi not in qtr_tiles:
                prefetch_qtr(gi, None)
            qtr_sb = qtr_tiles[gi]
            qt_sb = qtp.tile([PB, NKC * W], dt_s, tag="qt", name=f"qt{gi}")
            for kco in range(NKC):
                ps = psa.tile([PB, W], f32, tag="ps_a", name="psq")
                for kci in range(NKC):
                    MM(ps,
                       wq_sb[:, kco * DKEY + kci * PB:
                             kco * DKEY + (kci + 1) * PB],
                       qtr_sb[:, kci * W:(kci + 1) * W],
                       start=(kci == 0), stop=(kci == NKC - 1))
                nc.vector.tensor_copy(qt_sb[:, kco * W:(kco + 1) * W], ps)
            qt_tiles[gi] = qt_sb

        def attn_phase1(si):
            slot = slots[si]
            w, nbv = slot["w"], slot["nb"]
            W, qoff = gmeta[slot["gi"]]["W"], slot["qoff"]
            qt_sb = qt_tiles[slot["gi"]]
            mko = slot["mk_off"]

            vv_sb = vv_tiles.pop(si)
            p_sb = ptp.tile([PB, nbv * w], dt_s, tag="pt", name="pt")
            # running block-sum of the exps on the (otherwise idle) gpsimd
            # engine: phase 2 then needs ONE 1-col rowsum matmul per tile
            # instead of one per (s, tile) -- saves ~120 PE issue slots and
            # lets the reciprocal start before the P^T@V chain, not after
            psum_sb = (sump.tile([PB, w], dt_s, tag="sum", name="psum")
                       if nbv > 1 else None)
            for s in range(nbv):
                if si == 0:
                    kt_sb = kt_pre.pop(s)
                else:
                    kt_sb = dma_kt(si, s,
                                   nc.sync if s % 2 == 0 else nc.gpsimd)
                ps = psa.tile([PB, w], f32, tag="ps_a", name="pss")
                for kc in range(NKC):
                    MM(ps, kt_sb[:, kc * PB:(kc + 1) * PB],
                       qt_sb[:, kc * W + qoff:kc * W + qoff + w],
                       start=(kc == 0), stop=(kc == NKC - 1))
                nc.scalar.activation(p_sb[:, s * w:(s + 1) * w], ps, Exp,
                                     bias=mask_sb[:, mko + s:mko + s + 1],
                                     scale=SCALE)
                if s == 1:
                    nc.gpsimd.tensor_add(psum_sb, p_sb[:, 0:w],
                                         p_sb[:, w:2 * w])
                elif s > 1:
                    nc.gpsimd.tensor_add(psum_sb, psum_sb,
                                         p_sb[:, s * w:(s + 1) * w])
            if psum_sb is None:
                psum_sb = p_sb[:, 0:w]

            # next slot's V prefetch, gated on this slot's first exp
            if si + 1 < len(slots):
                dma_vv(si + 1, p_sb[:, 0:1])
            return p_sb, vv_sb, psum_sb

        def attn_phase2(si, p_sb, vv_sb, psum_sb):
            slot = slots[si]
            w, nbv = slot["w"], slot["nb"]
            for t in range(w // PB):
                po = pso.tile([PB, DV], f32, tag="ps_o", name="po")
                pr = psr.tile([PB, 1], f32, tag="ps_r", name="pr")
                # single rowsum matmul from the gpsimd block-sum, BEFORE
                # the chain: the reciprocal overlaps the whole P^T@V chain
                MM(pr, psum_sb[:, t * PB:(t + 1) * PB], ones_sb,
                   start=True, stop=True)
                for s in range(nbv):
                    ptt = p_sb[:, s * w + t * PB:s * w + (t + 1) * PB]
                    st, sp = s == 0, s == nbv - 1
                    MM(po[:, :512], ptt, vv_sb[:, s * DV:s * DV + 512],
                       start=st, stop=sp)
                    MM(po[:, 512:], ptt, vv_sb[:, s * DV + 512:(s + 1) * DV],
                       start=st, stop=sp)
                rcp = rcpp.tile([PB, 1], f32, tag="rcp", name="rcp")
                nc.vector.reciprocal(rcp, pr)
                o_sb = outsp.tile([PB, DV], dt_s, tag="outs", name="outs")
                last = (si == len(slots) - 1 and t == w // PB - 1)
                if not last:
                    nc.vector.tensor_scalar_mul(o_sb[:, :512], po[:, :512],
                                                rcp)
                    nc.vector.tensor_scalar_mul(o_sb[:, 512:], po[:, 512:],
                                                rcp)
                    # out stores ride the kt/vv queues, NOT scalar: an out
                    # DMA's in-queue wait on the normalize would delay the
                    # next slot's exps and stall its phase-1 chains
                    # last two slots: odd tiles store via scalar (its exps
                    # are done by then) so gpsimd's last DMA lands ~5us
                    # earlier and its software-DGE completion-detection
                    # DRAIN (~5.4us) overlaps compute instead of binding
                    # the final barrier
                    alt = nc.scalar if si >= len(slots) - 2 else nc.gpsimd
                    eng = nc.sync if t % 2 == 0 else alt
                    eng.dma_start(out=out_d[slot["tile0"] + t], in_=o_sb)
                else:
                    # final tile: normalize and store in quarters on the
                    # two HARDWARE-DGE queues (sync, scalar) so transfer
                    # overlaps normalize.  gpsimd is software-DGE: a final
                    # store there makes the epilogue GpSimd DRAIN wait
                    # ~4us extra for completion detection.
                    for q in range(4):
                        a, b = q * 256, (q + 1) * 256
                        nc.vector.tensor_scalar_mul(o_sb[:, a:b],
                                                    po[:, a:b], rcp)
                        eng = nc.sync if q % 2 == 0 else nc.scalar
                        eng.dma_start(out=out_d[slot["tile0"] + t][:, a:b],
                                      in_=o_sb[:, a:b])

        done_g = {0}
        qproj(0)
        # gated remainder of the first wave, released by qproj-0's last cast
        gate = qt_tiles[0][:, NKC * g0["W"] - 1:NKC * g0["W"]]
        for s in range(u0, nb0):
            kt_pre[s] = dma_kt(0, s, nc.sync if s % 2 == 0 else nc.gpsimd,
                               pre_src=gate)
        # qtr g1 BEFORE vv0: qproj(g1) runs right after phase1(0) and with
        # the earlier wq0 start its qtr landing margin was already ~zero;
        # vv0 is only consumed by phase2(0), one phase later
        if len(gmeta) > 1:
            prefetch_qtr(1, gate)
        dma_vv(0, gate)

        # depth-1 software pipeline: phase 1 of slot i+1 is emitted before
        # phase 2 of slot i, so every phase-2 chain consumes exp outputs
        # that are long since ready (kills slot-boundary stalls)
        pending = None
        for si, slot in enumerate(slots):
            if slot["gi"] not in done_g:
                done_g.add(slot["gi"])
                qproj(slot["gi"])
            ph1 = attn_phase1(si)
            if pending is not None:
                attn_phase2(*pending)
            pending = (si, *ph1)
        attn_phase2(*pending)

    nc.compile()
    return nc


def _core_binding(slots, c):
    """Per-core (batch, row0) for each slot."""
    out = []
    for s in slots:
        if len(s["batches"]) == 2:
            b = s["batches"][0] if c < 4 else s["batches"][1]
            row0 = (c % 4) * s["w"]
        else:
            b = s["batches"][0]
            row0 = c * s["w"]
        out.append((b, row0))
    return out


def _prepare(inputs, dtype):
    np_s = _np_store_dtype(dtype)
    queries = np.asarray(inputs["queries"], dtype=np.float32)
    keys = np.asarray(inputs["keys"], dtype=np.float32)
    values = np.asarray(inputs["values"], dtype=np.float32)
    valid_lens = np.asarray(inputs["valid_lens"]).astype(np.int64)
    W_q = np.asarray(inputs["W_q"], dtype=np.float32)

    nb = tuple(int(min(NSB, max(1, math.ceil(int(v) / PB)))) for v in valid_lens)
    slots, gmeta, tot = _build_schedule(nb)

    # wq[p, ko*1024 + ki*128 + j] = W_q[ki*128+p, ko*128+j]
    wqb = np.ascontiguousarray(
        W_q.reshape(NKC, PB, NKC, PB).transpose(1, 2, 0, 3)
        .reshape(PB, NKC * DKEY).astype(np_s))

    # per-batch f16 copies once
    keys_s = keys.astype(np_s)
    values_s = values.astype(np_s)
    queries_s = queries.astype(np_s)

    pos = np.arange(PB)
    in_maps = []
    for c in range(NCORES):
        bind = _core_binding(slots, c)
        qtr = np.empty((PB, NKC * tot["qtr"]), dtype=np_s)
        kt = np.empty((PB, tot["kt"]), dtype=np_s)
        vv = np.empty((PB, tot["vv"]), dtype=np_s)
        mk = np.empty((PB, tot["mk"]), dtype=np.float32)
        for si, s in enumerate(slots):
            b, row0 = bind[si]
            w, nbv = s["w"], s["nb"]
            # qtr: group-major, [kci][slot cols]
            g = gmeta[s["gi"]]
            qarr = queries_s[b, row0:row0 + w, :].reshape(w, NKC, PB)
            qarr = qarr.transpose(2, 1, 0)          # [p, kci, l]
            gb = NKC * g["base"]
            for kci in range(NKC):
                st = gb + kci * g["W"] + s["qoff"]
                qtr[:, st:st + w] = qarr[:, kci, :]
            # kt: block-major [s][kc][key]
            karr = keys_s[b, :nbv * PB, :].reshape(nbv, PB, NKC, PB)
            st = s["kt_off"]
            kt[:, st:st + nbv * NKC * PB] = (
                karr.transpose(3, 0, 2, 1).reshape(PB, nbv * NKC * PB))
            # vv: [s'][v]
            varr = values_s[b, :nbv * PB, :].reshape(nbv, PB, DV)
            vv[:, s["vv_off"]:s["vv_off"] + nbv * DV] = (
                varr.transpose(1, 0, 2).reshape(PB, nbv * DV))
            # mask columns
            kpos = (np.arange(nbv)[None, :] * PB + pos[:, None])
            mk[:, s["mk_off"]:s["mk_off"] + nbv] = np.where(
                kpos < int(valid_lens[b]), 0.0, MASK_NEG)
        in_maps.append({"qtr": qtr, "kt": kt, "vv": vv, "wq": wqb, "mk": mk})
    return nb, slots, in_maps


def _run(inputs, trace=False, dtype=None):
    _ensure_paths()
    from concourse.bass_utils import run_bass_kernel_spmd

    dtype = dtype or DTYPE
    nb, slots, in_maps = _prepare(inputs, dtype)
    key = (nb, dtype)
    if key not in _program_cache:
        _program_cache[key] = _build_program(nb, dtype)
    nc = _program_cache[key]

    core_ids = list(range(NCORES))
    res = run_bass_kernel_spmd(nc, in_maps, core_ids, trace=trace)
    global _last_results
    _last_results = res

    full = np.empty((B, LQ, DV), dtype=np.float32)
    for c in range(NCORES):
        oc = res.results[c]["out"]
        bind = _core_binding(slots, c)
        for si, s in enumerate(slots):
            b, row0 = bind[si]
            nt = s["w"] // PB
            full[b, row0:row0 + s["w"], :] = (
                oc[s["tile0"]:s["tile0"] + nt].reshape(s["w"], DV))
    return full, res.exec_time_ns


def kernel(**inputs) -> np.ndarray:
    return _run(inputs, trace=False)[0]



# revision 45
# speedup vs baseline: 1.0102x; 1.0102x over previous
"""Trainium2 Bass kernel for DiffDimDotProductAttention.

reference computation:
    q = queries @ W_q                      [B, LQ, DKEY]
    scores = q @ keys^T / sqrt(DKEY)       [B, LQ, LK]
    scores = where(arange(LK) < valid_len, scores, -1e6)
    attn = softmax(scores, axis=-1)
    out = attn @ values                    [B, LQ, DV]

Sharding: every core gets 2048 query rows. The schedule is a per-core list of
SLOTS, identical in shape across cores (SPMD: one program), but bound to
different (batch, row-range) data per core. A batch with nb active key blocks
contributes 8 x 256-row slices; when two batches share the same nb, each core
takes a single 512-row slice of ONE of them instead of 256 rows of both, so
that batch's K/V stream is fetched once per core instead of twice. For the
reference valid_lens this cuts K/V HBM traffic by 25% and lets the scores
matmuls stream 512 columns.

Per slot the device runs two phases (P = exp(masked scores/sqrt(d)) kept in
SBUF between them):
    phase 1: for each key block s: scoresT[s,l] chain over kc, then
             P[s,:] = exp(scores/32 + mask_col) on the scalar engine; the
             otherwise-idle gpsimd engine keeps a running block-sum
             psum = sum_s P[s,:] as the exps land
    phase 2: for each 128-row tile t: ONE 1-col rowsum matmul
             rowsum[t] = psum[:,t]^T @ ones first (so the reciprocal
             overlaps the whole chain), then chain over s of
             out[t] += P[s,t]^T @ V[s],
             then normalize (vector) and DMA the f16 tile out.
The gpsimd block-sum replaces the per-(s,t) 1-col rowsum matmuls of the
earlier version: ~120 fewer PE issue slots (~27ns each) and an earlier
reciprocal -> ~4.5us off the matmul stream.  The f16 block-sum adds
O(sqrt(nb)*eps_f16) ~ 1e-3 relative error to the denominators, well
inside the 2e-2 gate (measured total 5.4e-4).
Phase 2 needs only 2 PSUM banks per tile (double-buffered), so scores can
stream 512 wide, and each tile's normalize+store overlaps the next tile's
matmul chain.  Phases are software-pipelined depth-1 across slots (phase 1
of slot i+1 is emitted before phase 2 of slot i) so phase-2 chains never
wait on exp outputs, and the final tile stores in halves on two queues to
shorten the drain.  Softmax skips the max-subtraction: scores ~ N(0,1),
exp() cannot overflow, and softmax is shift invariant.  Masked lanes get
bias -1e6 and underflow to exactly 0.0.

DMA discipline (the hard-won part): every DMA issue queue (SP / Pool /
Act) has 8 completion semaphores, and a reader of DMA #p conservatively
waits for every use of semaphore p%8 scheduled before it.  An un-gated
DMA gets hoisted by the tile scheduler and so delays the prologue's
readers by its own completion time.  Therefore each queue carries exactly
8 un-gated prologue DMAs (W_q chunk 0 FIRST -- the first projection
LDWEIGHTS waits on it and it must head the sync queue, not sit behind
two qtr chunks -- then qtr group 0, the other W_q chunks, mask, the
first kt blocks of slot 0), and every later DMA is dependency-gated --
by pool-buffer exhaustion (kt per-block tiles, ~12 in flight across the
16 DMA engines) or an explicit 1-column pre-touch copy tied to the first
projection's output (slot-0 kt tail, group-1 qtr BEFORE slot-0 V: the
g1 projection follows phase1(0) directly and its landing margin is
~zero, while V is consumed one phase later) or to each slot's first exp
(next slot's V).  All DRAM layouts are contiguous per partition.  Out
stores ride the sync/gpsimd queues, not scalar, so their in-queue waits
never delay the next slot's exps -- EXCEPT the last two slots' odd
tiles, which store via scalar (its exps are done by then): gpsimd is
software-DGE and its epilogue DRAIN burns ~5.4us on completion
detection, so its last DMA must land early enough for the drain to
overlap compute instead of binding the final barrier.

Measured (2.37GHz warm PE): 197.1-199.3us vs 203.0 for the session-start
baseline; stream is gapless within ~3us of the 1-col/cycle f16 floor.
The ~13us tail is dominated by a framework-fixed ~6.5us semaphore-
zeroing epilogue plus the final stores and barrier.  The device flips
between 2.4GHz and a throttled 2.0GHz P0 state run-to-run (median
512-col MM spacing 215ns vs 258ns -- test.py prints a probe); compare
timings only at matched clock.
"""

import math
import os
import sys

import numpy as np

DTYPE = os.environ.get("KERNEL_DTYPE", "f16")


def _ensure_paths():
    try:
        import concourse  # noqa: F401
        return
    except ImportError:
        pass
    for p in (
        "/root/.axon_site",
        "/root/.axon_site/_ro/trn_rl_repo",
        "/root/.axon_site/_ro/pypackages",
        "/opt/trn_rl_repo",
    ):
        if p not in sys.path:
            sys.path.append(p)
    import concourse  # noqa: F401


B, LQ, LK, DQ, DKEY, DV = 8, 2048, 2048, 1024, 1024, 1024
NCORES = 8
SLICE = LQ // NCORES          # 256 query rows per batch-slice
PB = 128                      # partition block
NKC = DQ // PB                # 8 contraction blocks for the projections
NSB = LK // PB                # 16 key blocks
GW = 4                        # key blocks per kT DMA group
NWARM = 13                    # HAM warm-up matmuls (cover until the first
                              # real chain's data lands ~13us with the
                              # wq0-first wave; longer would queue-block
                              # the real chains behind the warmup)
SCALE = 1.0 / math.sqrt(DKEY)
MASK_NEG = -1.0e6

_program_cache = {}
_last_results = None


def _np_store_dtype(dtype):
    if dtype == "bf16":
        import ml_dtypes
        return ml_dtypes.bfloat16
    if dtype == "f16":
        return np.float16
    return np.float32


def _build_schedule(nb):
    """Identical-across-cores slot schedule from per-batch block counts.

    Returns (slots, groups). Each slot: dict with
      w: 256 or 512 query rows;  nb: active key blocks
      batches: 1 batch (every core takes its w-slice of it) or 2 equal-nb
               batches (cores 0-3 take 512-row slices of the first, 4-7 of
               the second)
      qoff: column offset of this slot inside its projection group
      gi: projection group index
    groups: list of dicts {slots: [slot indices], W: total width, base: qtr
            column base (in per-kc units)}.
    """
    order = sorted(range(B), key=lambda i: (-nb[i], i))
    byval = {}
    for b in order:
        byval.setdefault(nb[b], []).append(b)
    slots = []
    for v, bs in byval.items():
        i = 0
        while len(bs) - i >= 2:
            slots.append({"w": 2 * SLICE, "nb": v, "batches": (bs[i], bs[i + 1])})
            i += 2
        if i < len(bs):
            slots.append({"w": SLICE, "nb": v, "batches": (bs[i],)})
    # 512-row slots first (their 1.7us/block chains halve the K/V demand
    # rate, easing the cold start, and buy prefetch runway for the rest),
    # then 256-row slots; nb descending within each, smallest slot drains.
    slots.sort(key=lambda s: (-s["w"], -s["nb"]))

    groups = []
    pend = None
    for si, s in enumerate(slots):
        if s["w"] == 2 * SLICE:
            groups.append([si])
        elif pend is None:
            pend = si
        else:
            groups.append([pend, si])
            pend = None
    if pend is not None:
        groups.append([pend])

    base = 0
    gmeta = []
    for gi, g in enumerate(groups):
        W = sum(slots[si]["w"] for si in g)
        off = 0
        for si in g:
            slots[si]["qoff"] = off
            slots[si]["gi"] = gi
            off += slots[si]["w"]
        gmeta.append({"slots": g, "W": W, "base": base})
        base += W

    # flat offsets for kt / vv / mk DRAM params and out tiles
    kto = vvo = mko = to = 0
    for s in slots:
        s["kt_off"], s["vv_off"], s["mk_off"], s["tile0"] = kto, vvo, mko, to
        kto += NKC * s["nb"] * PB
        vvo += s["nb"] * DV
        mko += s["nb"]
        to += s["w"] // PB
    return slots, gmeta, {"kt": kto, "vv": vvo, "mk": mko, "tiles": to,
                          "qtr": base}


def _sig(nb):
    slots, gmeta, _ = _build_schedule(nb)
    return tuple((s["w"], s["nb"]) for s in slots)


def _build_program(nb, dtype):
    _ensure_paths()
    import concourse.mybir as mybir
    import concourse.tile as tile
    from concourse import bacc

    f32 = mybir.dt.float32
    if dtype == "bf16":
        dt_s = mybir.dt.bfloat16
    elif dtype == "f16":
        dt_s = mybir.dt.float16
    else:
        dt_s = mybir.dt.float32
    Exp = mybir.ActivationFunctionType.Exp
    Copy = mybir.ActivationFunctionType.Copy

    slots, gmeta, tot = _build_schedule(nb)
    # kt pool depth = first slot's block count, so the second slot's kt
    # DMAs are pool-gated behind the first slot's chains (un-gated DMAs
    # would be hoisted and poison reused prologue semaphores)
    kt_bufs = min(12, max(8, slots[0]["nb"]))

    nc = bacc.Bacc("TRN2", target_bir_lowering=False, debug=False,
                   num_devices=NCORES)
    # layouts are contiguous per partition for single-DMA loads
    qtr_d = nc.declare_dram_parameter("qtr", [PB, NKC * tot["qtr"]], dt_s,
                                      isOutput=False)
    kt_d = nc.declare_dram_parameter("kt", [PB, tot["kt"]], dt_s,
                                     isOutput=False)
    vv_d = nc.declare_dram_parameter("vv", [PB, tot["vv"]], dt_s,
                                     isOutput=False)
    wq_d = nc.declare_dram_parameter("wq", [PB, NKC * DKEY], dt_s,
                                     isOutput=False)
    mk_d = nc.declare_dram_parameter("mk", [PB, tot["mk"]], f32,
                                     isOutput=False)
    out_d = nc.declare_dram_parameter("out", [tot["tiles"], PB, DV], dt_s,
                                      isOutput=True)

    MM = nc.tensor.matmul

    with tile.TileContext(nc) as tc, \
         tc.tile_pool(name="const", bufs=1) as constp, \
         tc.tile_pool(name="qtr", bufs=2) as qtrp, \
         tc.tile_pool(name="qt", bufs=2) as qtp, \
         tc.tile_pool(name="kt", bufs=kt_bufs) as ktp, \
         tc.tile_pool(name="vt", bufs=3) as vtp, \
         tc.tile_pool(name="pt", bufs=2) as ptp, \
         tc.tile_pool(name="sum", bufs=2) as sump, \
         tc.tile_pool(name="outs", bufs=3) as outsp, \
         tc.tile_pool(name="rcp", bufs=4) as rcpp, \
         tc.tile_pool(name="ps_a", bufs=2, space="PSUM") as psa, \
         tc.tile_pool(name="ps_o", bufs=2, space="PSUM") as pso, \
         tc.tile_pool(name="ps_r", bufs=2, space="PSUM") as psr:

        ENGS = [nc.sync, nc.gpsimd, nc.scalar]
        wq_sb = constp.tile([PB, NKC * DKEY], dt_s)
        mask_sb = constp.tile([PB, tot["mk"]], f32)

        # Each DMA issue queue (SP / Pool / Act) has 8 completion
        # semaphores; a reader of DMA #p conservatively waits for every use
        # of semaphore p%8 scheduled before it, so any un-gated DMA past
        # position 8 delays the prologue's readers by its own completion.
        # Hence: at most 8 un-gated DMAs per queue (qtr0 + wq + mask +
        # first kt blocks), and every later DMA is dependency-gated via a
        # 1-column pre-touch copy or pool-buffer exhaustion.
        g0 = gmeta[0]
        g0c = NKC * g0["W"]
        qtr0_sb = qtrp.tile([PB, g0c], dt_s, tag="qtr", name="qtr0")

        kt_pre = {}     # s -> prefetched kt tile of slot 0
        vv_tiles = {}   # si -> vv tile

        def dma_kt(si, s, eng, pre_src=None):
            kt_sb = ktp.tile([PB, NKC * PB], dt_s, tag="kt", name="ktb")
            if pre_src is not None:
                nc.vector.tensor_copy(kt_sb[:, 0:1], pre_src)
            base = slots[si]["kt_off"] + s * NKC * PB
            eng.dma_start(out=kt_sb, in_=kt_d[:, base:base + NKC * PB])
            return kt_sb

        def dma_vv(si, pre_src):
            slot = slots[si]
            nbv, vo = slot["nb"], slot["vv_off"]
            vv_sb = vtp.tile([PB, nbv * DV], dt_s, tag="vt", name="vt")
            if pre_src is not None:
                nc.vector.tensor_copy(vv_sb[:, 0:1], pre_src)
            for ci, c in enumerate(range(0, nbv, 2)):
                e = min(c + 2, nbv)
                eng = nc.sync if ci % 2 == 0 else nc.gpsimd
                eng.dma_start(out=vv_sb[:, c * DV:e * DV],
                              in_=vv_d[:, vo + c * DV:vo + e * DV])
            vv_tiles[si] = vv_sb

        # prologue wave: exactly 8 un-gated DMAs per queue (fresh
        # semaphores).  qtr0 (6 chunks) + wq (8 kco chunks) + mask + the
        # first 9 kt blocks of slot 0 fill the 24 slots.  wq chunk 0 is
        # emitted FIRST (before the qtr chunks) so it heads the sync
        # queue: the very first projection LDWEIGHTS waits on it, and
        # behind two qtr chunks it landed ~3.5us later than necessary.
        nb0 = slots[0]["nb"]

        def dma_wq(kco):
            a, b = kco * DKEY, (kco + 1) * DKEY
            ENGS[kco % 3].dma_start(out=wq_sb[:, a:b], in_=wq_d[:, a:b])

        dma_wq(0)
        for i in range(6):
            a, b = g0c * i // 6, g0c * (i + 1) // 6
            ENGS[i % 3].dma_start(out=qtr0_sb[:, a:b], in_=qtr_d[:, a:b])
        for kco in range(1, NKC):
            dma_wq(kco)
        nc.scalar.dma_start(out=mask_sb, in_=mk_d[:, :])
        u0 = min(nb0, 9)
        for s in range(u0):
            kt_pre[s] = dma_kt(0, s, ENGS[s % 3])

        ones_sb = constp.tile([PB, 1], dt_s)
        nc.vector.memset(ones_sb, 1.0)

        # HAM warm-up: throwaway matmuls on zeroed SBUF while the first
        # DMA wave lands, so real work starts at full PE activity
        warm_sb = constp.tile([PB, 512], dt_s, name="warm_sb")
        nc.vector.memset(warm_sb, 0.0)
        wps = psa.tile([PB, 512], f32, tag="ps_a", name="warmps")
        for i in range(NWARM):
            MM(wps, warm_sb[:, :PB], warm_sb, start=(i == 0),
               stop=(i == NWARM - 1))

        qt_tiles = {}
        qtr_tiles = {0: qtr0_sb}

        def prefetch_qtr(gi, pre_src):
            g = gmeta[gi]
            gb, gc = NKC * g["base"], NKC * g["W"]
            qtr_sb = qtrp.tile([PB, gc], dt_s, tag="qtr", name="qtr")
            if pre_src is not None:
                nc.vector.tensor_copy(qtr_sb[:, 0:1], pre_src)
            for i in range(4):
                a, b = gc * i // 4, gc * (i + 1) // 4
                eng = nc.sync if i % 2 == 0 else nc.gpsimd
                eng.dma_start(out=qtr_sb[:, a:b], in_=qtr_d[:, gb + a:gb + b])
            qtr_tiles[gi] = qtr_sb

        def qproj(gi):
            g = gmeta[gi]
            W = g["W"]
            if gi not in qtr_tiles:
                prefetch_qtr(gi, None)
            qtr_sb = qtr_tiles[gi]
            qt_sb = qtp.tile([PB, NKC * W], dt_s, tag="qt", name=f"qt{gi}")
            for kco in range(NKC):
                ps = psa.tile([PB, W], f32, tag="ps_a", name="psq")
                for kci in range(NKC):
                    MM(ps,
                       wq_sb[:, kco * DKEY + kci * PB:
                             kco * DKEY + (kci + 1) * PB],
                       qtr_sb[:, kci * W:(kci + 1) * W],
                       start=(kci == 0), stop=(kci == NKC - 1))
                nc.vector.tensor_copy(qt_sb[:, kco * W:(kco + 1) * W], ps)
            qt_tiles[gi] = qt_sb

        def attn_phase1(si):
            slot = slots[si]
            w, nbv = slot["w"], slot["nb"]
            W, qoff = gmeta[slot["gi"]]["W"], slot["qoff"]
            qt_sb = qt_tiles[slot["gi"]]
            mko = slot["mk_off"]

            vv_sb = vv_tiles.pop(si)
            p_sb = ptp.tile([PB, nbv * w], dt_s, tag="pt", name="pt")
            # running block-sum of the exps on the (otherwise idle) gpsimd
            # engine: phase 2 then needs ONE 1-col rowsum matmul per tile
            # instead of one per (s, tile) -- saves ~120 PE issue slots and
            # lets the reciprocal start before the P^T@V chain, not after
            psum_sb = (sump.tile([PB, w], dt_s, tag="sum", name="psum")
                       if nbv > 1 else None)
            for s in range(nbv):
                if si == 0:
                    kt_sb = kt_pre.pop(s)
                else:
                    kt_sb = dma_kt(si, s,
                                   nc.sync if s % 2 == 0 else nc.gpsimd)
                ps = psa.tile([PB, w], f32, tag="ps_a", name="pss")
                for kc in range(NKC):
                    MM(ps, kt_sb[:, kc * PB:(kc + 1) * PB],
                       qt_sb[:, kc * W + qoff:kc * W + qoff + w],
                       start=(kc == 0), stop=(kc == NKC - 1))
                nc.scalar.activation(p_sb[:, s * w:(s + 1) * w], ps, Exp,
                                     bias=mask_sb[:, mko + s:mko + s + 1],
                                     scale=SCALE)
                if s == 1:
                    nc.gpsimd.tensor_add(psum_sb, p_sb[:, 0:w],
                                         p_sb[:, w:2 * w])
                elif s > 1:
                    nc.gpsimd.tensor_add(psum_sb, psum_sb,
                                         p_sb[:, s * w:(s + 1) * w])
            if psum_sb is None:
                psum_sb = p_sb[:, 0:w]

            # next slot's V prefetch, gated on this slot's first exp
            if si + 1 < len(slots):
                dma_vv(si + 1, p_sb[:, 0:1])
            return p_sb, vv_sb, psum_sb

        def attn_phase2(si, p_sb, vv_sb, psum_sb):
            slot = slots[si]
            w, nbv = slot["w"], slot["nb"]
            for t in range(w // PB):
                po = pso.tile([PB, DV], f32, tag="ps_o", name="po")
                pr = psr.tile([PB, 1], f32, tag="ps_r", name="pr")
                # single rowsum matmul from the gpsimd block-sum, BEFORE
                # the chain: the reciprocal overlaps the whole P^T@V chain
                MM(pr, psum_sb[:, t * PB:(t + 1) * PB], ones_sb,
                   start=True, stop=True)
                for s in range(nbv):
                    ptt = p_sb[:, s * w + t * PB:s * w + (t + 1) * PB]
                    st, sp = s == 0, s == nbv - 1
                    MM(po[:, :512], ptt, vv_sb[:, s * DV:s * DV + 512],
                       start=st, stop=sp)
                    MM(po[:, 512:], ptt, vv_sb[:, s * DV + 512:(s + 1) * DV],
                       start=st, stop=sp)
                rcp = rcpp.tile([PB, 1], f32, tag="rcp", name="rcp")
                nc.vector.reciprocal(rcp, pr)
                o_sb = outsp.tile([PB, DV], dt_s, tag="outs", name="outs")
                last = (si == len(slots) - 1 and t == w // PB - 1)
                if not last:
                    # one 1024-col normalize, not 2x512: the store is a
                    # single DMA that waits for the whole tile anyway, and
                    # each DVE op carries ~220ns fixed overhead -- on the
                    # last slot the vector turnaround paces the chains
                    nc.vector.tensor_scalar_mul(o_sb, po, rcp)
                    # out stores ride the kt/vv queues, NOT scalar: an out
                    # DMA's in-queue wait on the normalize would delay the
                    # next slot's exps and stall its phase-1 chains
                    # last two slots: odd tiles store via scalar (its exps
                    # are done by then) so gpsimd's last DMA lands ~5us
                    # earlier and its software-DGE completion-detection
                    # DRAIN (~5.4us) overlaps compute instead of binding
                    # the final barrier
                    alt = nc.scalar if si >= len(slots) - 2 else nc.gpsimd
                    eng = nc.sync if t % 2 == 0 else alt
                    eng.dma_start(out=out_d[slot["tile0"] + t], in_=o_sb)
                else:
                    # final tile: normalize and store in quarters on the
                    # two HARDWARE-DGE queues (sync, scalar) so transfer
                    # overlaps normalize.  gpsimd is software-DGE: a final
                    # store there makes the epilogue GpSimd DRAIN wait
                    # ~4us extra for completion detection.
                    for q in range(4):
                        a, b = q * 256, (q + 1) * 256
                        nc.vector.tensor_scalar_mul(o_sb[:, a:b],
                                                    po[:, a:b], rcp)
                        eng = nc.sync if q % 2 == 0 else nc.scalar
                        eng.dma_start(out=out_d[slot["tile0"] + t][:, a:b],
                                      in_=o_sb[:, a:b])

        done_g = {0}
        qproj(0)
        # gated remainder of the first wave, released by qproj-0's last cast
        gate = qt_tiles[0][:, NKC * g0["W"] - 1:NKC * g0["W"]]
        for s in range(u0, nb0):
            kt_pre[s] = dma_kt(0, s, nc.sync if s % 2 == 0 else nc.gpsimd,
                               pre_src=gate)
        # qtr g1 BEFORE vv0: qproj(g1) runs right after phase1(0) and with
        # the earlier wq0 start its qtr landing margin was already ~zero;
        # vv0 is only consumed by phase2(0), one phase later
        if len(gmeta) > 1:
            prefetch_qtr(1, gate)
        dma_vv(0, gate)

        # depth-1 software pipeline: phase 1 of slot i+1 is emitted before
        # phase 2 of slot i, so every phase-2 chain consumes exp outputs
        # that are long since ready (kills slot-boundary stalls)
        pending = None
        for si, slot in enumerate(slots):
            if slot["gi"] not in done_g:
                done_g.add(slot["gi"])
                qproj(slot["gi"])
            ph1 = attn_phase1(si)
            if pending is not None:
                attn_phase2(*pending)
            pending = (si, *ph1)
        attn_phase2(*pending)

    nc.compile()
    return nc


def _core_binding(slots, c):
    """Per-core (batch, row0) for each slot."""
    out = []
    for s in slots:
        if len(s["batches"]) == 2:
            b = s["batches"][0] if c < 4 else s["batches"][1]
            row0 = (c % 4) * s["w"]
        else:
            b = s["batches"][0]
            row0 = c * s["w"]
        out.append((b, row0))
    return out


def _prepare(inputs, dtype):
    np_s = _np_store_dtype(dtype)
    queries = np.asarray(inputs["queries"], dtype=np.float32)
    keys = np.asarray(inputs["keys"], dtype=np.float32)
    values = np.asarray(inputs["values"], dtype=np.float32)
    valid_lens = np.asarray(inputs["valid_lens"]).astype(np.int64)
    W_q = np.asarray(inputs["W_q"], dtype=np.float32)

    nb = tuple(int(min(NSB, max(1, math.ceil(int(v) / PB)))) for v in valid_lens)
    slots, gmeta, tot = _build_schedule(nb)

    # wq[p, ko*1024 + ki*128 + j] = W_q[ki*128+p, ko*128+j]
    wqb = np.ascontiguousarray(
        W_q.reshape(NKC, PB, NKC, PB).transpose(1, 2, 0, 3)
        .reshape(PB, NKC * DKEY).astype(np_s))

    # per-batch f16 copies once
    keys_s = keys.astype(np_s)
    values_s = values.astype(np_s)
    queries_s = queries.astype(np_s)

    pos = np.arange(PB)
    in_maps = []
    for c in range(NCORES):
        bind = _core_binding(slots, c)
        qtr = np.empty((PB, NKC * tot["qtr"]), dtype=np_s)
        kt = np.empty((PB, tot["kt"]), dtype=np_s)
        vv = np.empty((PB, tot["vv"]), dtype=np_s)
        mk = np.empty((PB, tot["mk"]), dtype=np.float32)
        for si, s in enumerate(slots):
            b, row0 = bind[si]
            w, nbv = s["w"], s["nb"]
            # qtr: group-major, [kci][slot cols]
            g = gmeta[s["gi"]]
            qarr = queries_s[b, row0:row0 + w, :].reshape(w, NKC, PB)
            qarr = qarr.transpose(2, 1, 0)          # [p, kci, l]
            gb = NKC * g["base"]
            for kci in range(NKC):
                st = gb + kci * g["W"] + s["qoff"]
                qtr[:, st:st + w] = qarr[:, kci, :]
            # kt: block-major [s][kc][key]
            karr = keys_s[b, :nbv * PB, :].reshape(nbv, PB, NKC, PB)
            st = s["kt_off"]
            kt[:, st:st + nbv * NKC * PB] = (
                karr.transpose(3, 0, 2, 1).reshape(PB, nbv * NKC * PB))
            # vv: [s'][v]
            varr = values_s[b, :nbv * PB, :].reshape(nbv, PB, DV)
            vv[:, s["vv_off"]:s["vv_off"] + nbv * DV] = (
                varr.transpose(1, 0, 2).reshape(PB, nbv * DV))
            # mask columns
            kpos = (np.arange(nbv)[None, :] * PB + pos[:, None])
            mk[:, s["mk_off"]:s["mk_off"] + nbv] = np.where(
                kpos < int(valid_lens[b]), 0.0, MASK_NEG)
        in_maps.append({"qtr": qtr, "kt": kt, "vv": vv, "wq": wqb, "mk": mk})
    return nb, slots, in_maps


def _run(inputs, trace=False, dtype=None):
    _ensure_paths()
    from concourse.bass_utils import run_bass_kernel_spmd

    dtype = dtype or DTYPE
    nb, slots, in_maps = _prepare(inputs, dtype)
    key = (nb, dtype)
    if key not in _program_cache:
        _program_cache[key] = _build_program(nb, dtype)
    nc = _program_cache[key]

    core_ids = list(range(NCORES))
    res = run_bass_kernel_spmd(nc, in_maps, core_ids, trace=trace)
    global _last_results
    _last_results = res

    full = np.empty((B, LQ, DV), dtype=np.float32)
    for c in range(NCORES):
        oc = res.results[c]["out"]
        bind = _core_binding(slots, c)
        for si, s in enumerate(slots):
            b, row0 = bind[si]
            nt = s["w"] // PB
            full[b, row0:row0 + s["w"], :] = (
                oc[s["tile0"]:s["tile0"] + nt].reshape(s["w"], DV))
    return full, res.exec_time_ns


def kernel(**inputs) -> np.ndarray:
    return _run(inputs, trace=False)[0]

